# revision 21
# baseline (speedup 1.0000x reference)
"""WOQ int4 linear (4096x4096x11008, G=128) + bias + silu + mul on 8 NeuronCores.

Column-parallel: N=11008 is split into 8 shards of 1376; inp is replicated.
Per core: dequantize the int4 weight shard to bf16 in SBUF once, transpose
inp tiles on the tensor engine, then a bf16 matmul accumulating in PSUM with
the bias folded in as a rank-1 matmul, and a fused silu*mul epilogue.

kernel(**inputs) takes the FULL inputs (as produced by the problem's
setup_inputs) and returns the FULL [4096, 11008] float32 output.
"""

import sys
from contextlib import ExitStack

if "/opt/trn_rl_repo" not in sys.path:
    sys.path.insert(0, "/opt/trn_rl_repo")

import numpy as np

import concourse.bacc as bacc
import concourse.mybir as mybir
import concourse.tile as tile
from concourse.bass_utils import run_bass_kernel_spmd
from concourse.masks import make_identity

FP32 = mybir.dt.float32
BF16 = mybir.dt.bfloat16
I32 = mybir.dt.int32
I16 = mybir.dt.int16

M, K, N = 4096, 4096, 11008
NCORES = 8
NS = N // NCORES          # 1376 columns per core
NW = NS // 8              # 172 packed int32 words per row
G = 128


def _n_chunks(NS, step=512):
    out, off = [], 0
    while off < NS:
        ln = min(step, NS - off)
        out.append((off, ln))
        off += ln
    return out


def _build_kernel(ctx: ExitStack, tc: tile.TileContext, io: dict):
    nc = tc.nc
    inp, qweight, scales, qzeros, bias, mul, out = (
        io["inp"], io["qweight"], io["woq_scales"], io["woq_qzeros"],
        io["woq_bias"], io["mul"], io["out"],
    )
    M, K = inp.shape
    NS = out.shape[1]
    NW = NS // 8
    KT = K // 128             # k-tiles == quant groups (G == 128)
    MT = M // 128
    KH = K // 4               # inp is staged in row quarters
    CH = _n_chunks(NS)

    const = ctx.enter_context(tc.tile_pool(name="const", bufs=1))
    wpool = ctx.enter_context(tc.tile_pool(name="wpool", bufs=KT // 2))
    prep = ctx.enter_context(tc.tile_pool(name="prep", bufs=1))
    deq = ctx.enter_context(tc.tile_pool(name="deq", bufs=2))
    bcast = ctx.enter_context(tc.tile_pool(name="bcast", bufs=3))
    qwpool = ctx.enter_context(tc.tile_pool(name="qwpool", bufs=2))
    mloop = ctx.enter_context(tc.tile_pool(name="mloop", bufs=3))
    ep = ctx.enter_context(tc.tile_pool(name="ep", bufs=2))
    inpool = ctx.enter_context(tc.tile_pool(name="inpool", bufs=2))
    dram = ctx.enter_context(tc.tile_pool(name="dram", bufs=1, space="DRAM"))
    psum_mm_pool = ctx.enter_context(tc.tile_pool(name="psmm", bufs=2, space="PSUM"))
    psum_tp_pool = ctx.enter_context(tc.tile_pool(name="pstp", bufs=2, space="PSUM"))

    # constants
    identity = const.tile([128, 128], FP32)
    make_identity(nc, identity)
    ones_col = const.tile([1, 128], BF16)
    nc.vector.memset(ones_col[:], 1.0)

    # ---- phase A: zeros/scales prep ----
    s_sb = prep.tile([KT, NS], FP32)
    nc.sync.dma_start(s_sb[:], scales[:, :])
    qz_sb = prep.tile([KT, NW], I32)
    nc.sync.dma_start(qz_sb[:], qzeros[:, :])
    bias_bf = prep.tile([1, NS], BF16)
    nc.gpsimd.dma_start(bias_bf[:], bias[:, :])

    z_i32 = deq.tile([KT, NW, 8], I32, tag="wint")
    for j in range(8):
        nc.vector.tensor_scalar(
            out=z_i32[:, :, j], in0=qz_sb[:],
            scalar1=4 * j, scalar2=0xF,
            op0=mybir.AluOpType.logical_shift_right,
            op1=mybir.AluOpType.bitwise_and,
        )
    zs = deq.tile([KT, NS], FP32, tag="wint", name="zs")
    nc.vector.tensor_copy(zs[:], z_i32.rearrange("p a b -> p (a b)"))
    nc.vector.tensor_tensor(
        out=zs[:], in0=zs[:], in1=s_sb[:], op=mybir.AluOpType.mult
    )
    s_bf = prep.tile([KT, NS], BF16)
    nc.vector.tensor_copy(s_bf[:], s_sb[:])
    zs_bf = prep.tile([KT, NS], BF16)
    nc.vector.tensor_copy(zs_bf[:], zs[:])
    s_d = dram.tile([KT, NS], BF16)
    nc.sync.dma_start(s_d[:], s_bf[:])
    zs_d = dram.tile([KT, NS], BF16)
    nc.sync.dma_start(zs_d[:], zs_bf[:])

    # ---- phase B: dequantize weight shard into SBUF, bf16, [k_p, n_f] ----
    # w[k, n] = wint[k, n] * s[g, n] - z[g, n] * s[g, n]; two groups per pass
    KP = KT // 2
    w_sb2 = [wpool.tile([128, 2, NS], BF16, tag="w", name=f"w_sb{i}") for i in range(KP)]
    for g2 in range(KP):
        s_b = bcast.tile([128, 2, NS], BF16, tag="s_b")
        zs_b = bcast.tile([128, 2, NS], BF16, tag="zs_b")
        for a in range(2):
            g = 2 * g2 + a
            nc.sync.dma_start(s_b[:, a, :], s_d[g : g + 1, :].broadcast_to([128, NS]))
            nc.sync.dma_start(zs_b[:, a, :], zs_d[g : g + 1, :].broadcast_to([128, NS]))

        qw_t = qwpool.tile([128, 2, NW], I32, tag="qw")
        nc.sync.dma_start(
            qw_t[:],
            qweight[g2 * 256 : (g2 + 1) * 256, :].rearrange("(a p) w -> p a w", p=128),
        )
        wint = deq.tile([128, 2, NW, 8], I32, tag="wint")
        for j in range(8):
            nc.vector.tensor_scalar(
                out=wint[:, :, :, j], in0=qw_t[:],
                scalar1=4 * j, scalar2=0xF,
                op0=mybir.AluOpType.logical_shift_right,
                op1=mybir.AluOpType.bitwise_and,
            )
        wf = w_sb2[g2].rearrange("p a b -> p (a b)")
        nc.vector.tensor_tensor(
            out=wf, in0=wint.rearrange("p a b c -> p (a b c)"),
            in1=s_b.rearrange("p a b -> p (a b)"), op=mybir.AluOpType.mult,
        )
        nc.vector.tensor_tensor(
            out=wf, in0=wf, in1=zs_b.rearrange("p a b -> p (a b)"),
            op=mybir.AluOpType.subtract,
        )

    # ---- phase C: m-tile pairs: transpose inp, interleaved matmuls, epilogue ----
    assert MT % 2 == 0
    for mp in range(0, MT, 2):
        inpTs = []
        for mi in (mp, mp + 1):
            inpT = mloop.tile([128, KT, 128], BF16, tag="inpT", name=f"inpT{mi}")
            inpTs.append(inpT)
            for half in range(4):
                inp_t = inpool.tile([128, KH], FP32, tag="inp")
                nc.scalar.dma_start(
                    inp_t[:], inp[mi * 128 : (mi + 1) * 128, half * KH : (half + 1) * KH]
                )
                kt0 = half * (KT // 4)
                for kt4 in range(0, KT // 4, 4):
                    ng = min(4, KT // 4 - kt4)
                    ps_tp = psum_tp_pool.tile([128, 512], FP32, tag="tp")
                    for q in range(ng):
                        nc.tensor.transpose(
                            ps_tp[:, q * 128 : (q + 1) * 128],
                            inp_t[:, (kt4 + q) * 128 : (kt4 + q + 1) * 128],
                            identity[:],
                        )
                    dst = inpT[:, kt0 + kt4 : kt0 + kt4 + ng, :].rearrange("p a b -> p (a b)")
                    nc.scalar.copy(dst, ps_tp[:, : ng * 128])

        pss = []
        for mi in (mp, mp + 1):
            ps = psum_mm_pool.tile([128, NS], FP32, tag="mm", name=f"ps{mi}")
            pss.append(ps)
            for off, ln in CH:
                nc.tensor.matmul(
                    ps[:, off : off + ln], ones_col[:], bias_bf[:, off : off + ln],
                    start=True, stop=False,
                )
        for kt in range(KT):
            last = kt == KT - 1
            for ps, inpT in zip(pss, inpTs):
                for off, ln in CH:
                    nc.tensor.matmul(
                        ps[:, off : off + ln],
                        inpT[:, kt, :],
                        w_sb2[kt // 2][:, kt % 2, off : off + ln],
                        start=False, stop=last,
                    )

        for ps, mi in zip(pss, (mp, mp + 1)):
            mul_t = ep.tile([128, NS], FP32, tag="mul")
            nc.scalar.dma_start(mul_t[:], mul[mi * 128 : (mi + 1) * 128, :])
            silu_t = ep.tile([128, NS], BF16, tag="silu")
            nc.scalar.activation(silu_t[:], ps[:], mybir.ActivationFunctionType.Silu)
            nc.vector.tensor_tensor(
                out=mul_t[:], in0=silu_t[:], in1=mul_t[:], op=mybir.AluOpType.mult
            )
            nc.sync.dma_start(out[mi * 128 : (mi + 1) * 128, :], mul_t[:])


_NC_CACHE = None


def _get_nc():
    global _NC_CACHE
    if _NC_CACHE is not None:
        return _NC_CACHE
    nc = bacc.Bacc("TRN2", target_bir_lowering=False, debug=False, num_devices=NCORES)
    KT = K // 128
    io = {
        "inp": nc.dram_tensor("inp", [M, K], FP32, kind="ExternalInput").ap(),
        "qweight": nc.dram_tensor("qweight", [K, NW], I32, kind="ExternalInput").ap(),
        "woq_scales": nc.dram_tensor("woq_scales", [KT, NS], FP32, kind="ExternalInput").ap(),
        "woq_qzeros": nc.dram_tensor("woq_qzeros", [KT, NW], I32, kind="ExternalInput").ap(),
        "woq_bias": nc.dram_tensor("woq_bias", [1, NS], FP32, kind="ExternalInput").ap(),
        "mul": nc.dram_tensor("mul", [M, NS], FP32, kind="ExternalInput").ap(),
        "out": nc.dram_tensor("out", [M, NS], FP32, kind="ExternalOutput").ap(),
    }
    with tile.TileContext(nc) as tc:
        with ExitStack() as ctx:
            _build_kernel(ctx, tc, io)
    nc.compile()
    _NC_CACHE = nc
    return nc


def _shard_inputs(inp, qweight, woq_scales, woq_qzeros, woq_bias, mul):
    in_maps = []
    for c in range(NCORES):
        n0, nw0 = c * NS, c * NW
        in_maps.append({
            "inp": inp,
            "qweight": np.ascontiguousarray(qweight[:, nw0 : nw0 + NW]),
            "woq_scales": np.ascontiguousarray(woq_scales[:, n0 : n0 + NS]),
            "woq_qzeros": np.ascontiguousarray(woq_qzeros[:, nw0 : nw0 + NW]),
            "woq_bias": np.ascontiguousarray(woq_bias.reshape(1, N)[:, n0 : n0 + NS]),
            "mul": np.ascontiguousarray(mul[:, n0 : n0 + NS]),
        })
    return in_maps


def run(inputs: dict, trace: bool = False):
    inp = np.asarray(inputs["inp"], dtype=np.float32)
    qweight = np.asarray(inputs["qweight"], dtype=np.int32)
    woq_scales = np.asarray(inputs["woq_scales"], dtype=np.float32)
    woq_qzeros = np.asarray(inputs["woq_qzeros"], dtype=np.int32)
    woq_bias = np.asarray(inputs["woq_bias"], dtype=np.float32)
    mul = np.asarray(inputs["mul"], dtype=np.float32)
    assert int(inputs.get("group_size", G)) == G
    assert inp.shape == (M, K) and qweight.shape == (K, N // 8)

    nc = _get_nc()
    in_maps = _shard_inputs(inp, qweight, woq_scales, woq_qzeros, woq_bias, mul)
    res = run_bass_kernel_spmd(nc, in_maps, core_ids=list(range(NCORES)), trace=trace)
    out = np.empty((M, N), dtype=np.float32)
    for c in range(NCORES):
        out[:, c * NS : (c + 1) * NS] = res.results[c]["out"]
    return out, res


def kernel(**inputs) -> np.ndarray:
    out, _ = run(inputs, trace=False)
    return out


# revision 22
# speedup vs baseline: 1.0267x; 1.0267x over previous
"""WOQ int4 linear (4096x4096x11008, G=128) + bias + silu + mul on 8 NeuronCores.

Column-parallel: N=11008 is split into 8 shards of 1376; inp is replicated.
Per core: dequantize the int4 weight shard to bf16 in SBUF once, transpose
inp tiles on the tensor engine, then a bf16 matmul accumulating in PSUM with
the bias folded in as a rank-1 matmul, and a fused silu*mul epilogue.

kernel(**inputs) takes the FULL inputs (as produced by the problem's
setup_inputs) and returns the FULL [4096, 11008] float32 output.
"""

import sys
from contextlib import ExitStack

if "/opt/trn_rl_repo" not in sys.path:
    sys.path.insert(0, "/opt/trn_rl_repo")

import numpy as np

import concourse.bacc as bacc
import concourse.mybir as mybir
import concourse.tile as tile
from concourse.bass_utils import run_bass_kernel_spmd
from concourse.masks import make_identity

FP32 = mybir.dt.float32
BF16 = mybir.dt.bfloat16
I32 = mybir.dt.int32
I16 = mybir.dt.int16

M, K, N = 4096, 4096, 11008
NCORES = 8
NS = N // NCORES          # 1376 columns per core
NW = NS // 8              # 172 packed int32 words per row
G = 128


def _n_chunks(NS, step=512):
    out, off = [], 0
    while off < NS:
        ln = min(step, NS - off)
        out.append((off, ln))
        off += ln
    return out


def _build_kernel(ctx: ExitStack, tc: tile.TileContext, io: dict):
    nc = tc.nc
    inp, qweight, scales, qzeros, bias, mul, out = (
        io["inp"], io["qweight"], io["woq_scales"], io["woq_qzeros"],
        io["woq_bias"], io["mul"], io["out"],
    )
    M, K = inp.shape
    NS = out.shape[1]
    NW = NS // 8
    KT = K // 128             # k-tiles == quant groups (G == 128)
    MT = M // 128
    KH = K // 2               # inp is staged in row halves
    CH = _n_chunks(NS)

    const = ctx.enter_context(tc.tile_pool(name="const", bufs=1))
    wpool = ctx.enter_context(tc.tile_pool(name="wpool", bufs=KT // 2))
    prep = ctx.enter_context(tc.tile_pool(name="prep", bufs=1))
    deq = ctx.enter_context(tc.tile_pool(name="deq", bufs=2))
    bcast = ctx.enter_context(tc.tile_pool(name="bcast", bufs=1))
    qwpool = ctx.enter_context(tc.tile_pool(name="qwpool", bufs=2))
    mloop = ctx.enter_context(tc.tile_pool(name="mloop", bufs=3))
    ep = ctx.enter_context(tc.tile_pool(name="ep", bufs=2))
    inpool = ctx.enter_context(tc.tile_pool(name="inpool", bufs=2))
    dram = ctx.enter_context(tc.tile_pool(name="dram", bufs=1, space="DRAM"))
    psum_mm_pool = ctx.enter_context(tc.tile_pool(name="psmm", bufs=6, space="PSUM"))
    psum_tp_pool = ctx.enter_context(tc.tile_pool(name="pstp", bufs=2, space="PSUM"))

    # constants
    identity = const.tile([128, 128], FP32)
    make_identity(nc, identity)
    ones_col = const.tile([1, 128], BF16)
    nc.vector.memset(ones_col[:], 1.0)

    # ---- phase A: zeros/scales prep ----
    s_sb = prep.tile([KT, NS], FP32)
    nc.sync.dma_start(s_sb[:], scales[:, :])
    qz_sb = prep.tile([KT, NW], I32)
    nc.sync.dma_start(qz_sb[:], qzeros[:, :])
    bias_bf = prep.tile([1, NS], BF16)
    nc.gpsimd.dma_start(bias_bf[:], bias[:, :])

    z_i32 = deq.tile([KT, NW, 8], I32, tag="wint")
    for j in range(8):
        nc.vector.tensor_scalar(
            out=z_i32[:, :, j], in0=qz_sb[:],
            scalar1=4 * j, scalar2=0xF,
            op0=mybir.AluOpType.logical_shift_right,
            op1=mybir.AluOpType.bitwise_and,
        )
    zs = deq.tile([KT, NS], FP32, tag="wint", name="zs")
    nc.vector.tensor_copy(zs[:], z_i32.rearrange("p a b -> p (a b)"))
    nc.vector.tensor_tensor(
        out=zs[:], in0=zs[:], in1=s_sb[:], op=mybir.AluOpType.mult
    )
    s_bf = prep.tile([KT, NS], BF16)
    nc.vector.tensor_copy(s_bf[:], s_sb[:])
    zs_bf = prep.tile([KT, NS], BF16)
    nc.vector.tensor_copy(zs_bf[:], zs[:])
    s_d = dram.tile([KT, NS], BF16)
    nc.sync.dma_start(s_d[:], s_bf[:])
    zs_d = dram.tile([KT, NS], BF16)
    nc.sync.dma_start(zs_d[:], zs_bf[:])

    # ---- phase B: dequantize weight shard into SBUF, bf16, [k_p, n_f] ----
    # w[k, n] = wint[k, n] * s[g, n] - z[g, n] * s[g, n]; two groups per pass
    KP = KT // 2
    w_sb2 = [wpool.tile([128, 2, NS], BF16, tag="w", name=f"w_sb{i}") for i in range(KP)]
    for g2 in range(KP):
        s_b = bcast.tile([128, 2, NS], BF16, tag="s_b")
        zs_b = bcast.tile([128, 2, NS], BF16, tag="zs_b")
        for a in range(2):
            g = 2 * g2 + a
            nc.sync.dma_start(s_b[:, a, :], s_d[g : g + 1, :].broadcast_to([128, NS]))
            nc.sync.dma_start(zs_b[:, a, :], zs_d[g : g + 1, :].broadcast_to([128, NS]))

        qw_t = qwpool.tile([128, 2, NW], I32, tag="qw")
        nc.sync.dma_start(
            qw_t[:],
            qweight[g2 * 256 : (g2 + 1) * 256, :].rearrange("(a p) w -> p a w", p=128),
        )
        wint = deq.tile([128, 2, NW, 8], I32, tag="wint")
        for j in range(8):
            nc.vector.tensor_scalar(
                out=wint[:, :, :, j], in0=qw_t[:],
                scalar1=4 * j, scalar2=0xF,
                op0=mybir.AluOpType.logical_shift_right,
                op1=mybir.AluOpType.bitwise_and,
            )
        wf = w_sb2[g2].rearrange("p a b -> p (a b)")
        nc.vector.tensor_tensor(
            out=wf, in0=wint.rearrange("p a b c -> p (a b c)"),
            in1=s_b.rearrange("p a b -> p (a b)"), op=mybir.AluOpType.mult,
        )
        nc.vector.tensor_tensor(
            out=wf, in0=wf, in1=zs_b.rearrange("p a b -> p (a b)"),
            op=mybir.AluOpType.subtract,
        )

    # ---- phase C: m-tile pairs: transpose inp, interleaved matmuls, epilogue ----
    assert MT % 2 == 0
    for mp in range(0, MT, 2):
        inpTs = []
        for mi in (mp, mp + 1):
            inpT = mloop.tile([128, KT, 128], BF16, tag="inpT", name=f"inpT{mi}")
            inpTs.append(inpT)
            for half in range(2):
                inp_t = inpool.tile([128, KH], FP32, tag="inp")
                nc.scalar.dma_start(
                    inp_t[:], inp[mi * 128 : (mi + 1) * 128, half * KH : (half + 1) * KH]
                )
                kt0 = half * (KT // 2)
                for kt4 in range(0, KT // 2, 4):
                    ng = min(4, KT // 2 - kt4)
                    ps_tp = psum_tp_pool.tile([128, 512], FP32, tag="tp")
                    for q in range(ng):
                        nc.tensor.transpose(
                            ps_tp[:, q * 128 : (q + 1) * 128],
                            inp_t[:, (kt4 + q) * 128 : (kt4 + q + 1) * 128],
                            identity[:],
                        )
                    dst = inpT[:, kt0 + kt4 : kt0 + kt4 + ng, :].rearrange("p a b -> p (a b)")
                    nc.scalar.copy(dst, ps_tp[:, : ng * 128])

        pss = []
        for mi in (mp, mp + 1):
            row = []
            for ci, (off, ln) in enumerate(CH):
                pc = psum_mm_pool.tile([128, 512], FP32, tag="mm", name=f"ps{mi}_{ci}")
                row.append(pc)
                nc.tensor.matmul(
                    pc[:, :ln], ones_col[:], bias_bf[:, off : off + ln],
                    start=True, stop=False,
                )
            pss.append(row)
        for kt in range(KT):
            last = kt == KT - 1
            for row, inpT in zip(pss, inpTs):
                for ci, (off, ln) in enumerate(CH):
                    nc.tensor.matmul(
                        row[ci][:, :ln],
                        inpT[:, kt, :],
                        w_sb2[kt // 2][:, kt % 2, off : off + ln],
                        start=False, stop=last,
                    )

        for row, mi in zip(pss, (mp, mp + 1)):
            mul_t = ep.tile([128, NS], FP32, tag="mul")
            nc.scalar.dma_start(mul_t[:], mul[mi * 128 : (mi + 1) * 128, :])
            silu_t = ep.tile([128, NS], BF16, tag="silu")
            for ci, (off, ln) in enumerate(CH):
                nc.scalar.activation(
                    silu_t[:, off : off + ln], row[ci][:, :ln],
                    mybir.ActivationFunctionType.Silu,
                )
                nc.vector.tensor_tensor(
                    out=mul_t[:, off : off + ln], in0=silu_t[:, off : off + ln],
                    in1=mul_t[:, off : off + ln], op=mybir.AluOpType.mult,
                )
            nc.sync.dma_start(out[mi * 128 : (mi + 1) * 128, :], mul_t[:])


_NC_CACHE = None


def _get_nc():
    global _NC_CACHE
    if _NC_CACHE is not None:
        return _NC_CACHE
    nc = bacc.Bacc("TRN2", target_bir_lowering=False, debug=False, num_devices=NCORES)
    KT = K // 128
    io = {
        "inp": nc.dram_tensor("inp", [M, K], FP32, kind="ExternalInput").ap(),
        "qweight": nc.dram_tensor("qweight", [K, NW], I32, kind="ExternalInput").ap(),
        "woq_scales": nc.dram_tensor("woq_scales", [KT, NS], FP32, kind="ExternalInput").ap(),
        "woq_qzeros": nc.dram_tensor("woq_qzeros", [KT, NW], I32, kind="ExternalInput").ap(),
        "woq_bias": nc.dram_tensor("woq_bias", [1, NS], FP32, kind="ExternalInput").ap(),
        "mul": nc.dram_tensor("mul", [M, NS], FP32, kind="ExternalInput").ap(),
        "out": nc.dram_tensor("out", [M, NS], FP32, kind="ExternalOutput").ap(),
    }
    with tile.TileContext(nc) as tc:
        with ExitStack() as ctx:
            _build_kernel(ctx, tc, io)
    nc.compile()
    _NC_CACHE = nc
    return nc


def _shard_inputs(inp, qweight, woq_scales, woq_qzeros, woq_bias, mul):
    in_maps = []
    for c in range(NCORES):
        n0, nw0 = c * NS, c * NW
        in_maps.append({
            "inp": inp,
            "qweight": np.ascontiguousarray(qweight[:, nw0 : nw0 + NW]),
            "woq_scales": np.ascontiguousarray(woq_scales[:, n0 : n0 + NS]),
            "woq_qzeros": np.ascontiguousarray(woq_qzeros[:, nw0 : nw0 + NW]),
            "woq_bias": np.ascontiguousarray(woq_bias.reshape(1, N)[:, n0 : n0 + NS]),
            "mul": np.ascontiguousarray(mul[:, n0 : n0 + NS]),
        })
    return in_maps


def run(inputs: dict, trace: bool = False):
    inp = np.asarray(inputs["inp"], dtype=np.float32)
    qweight = np.asarray(inputs["qweight"], dtype=np.int32)
    woq_scales = np.asarray(inputs["woq_scales"], dtype=np.float32)
    woq_qzeros = np.asarray(inputs["woq_qzeros"], dtype=np.int32)
    woq_bias = np.asarray(inputs["woq_bias"], dtype=np.float32)
    mul = np.asarray(inputs["mul"], dtype=np.float32)
    assert int(inputs.get("group_size", G)) == G
    assert inp.shape == (M, K) and qweight.shape == (K, N // 8)

    nc = _get_nc()
    in_maps = _shard_inputs(inp, qweight, woq_scales, woq_qzeros, woq_bias, mul)
    res = run_bass_kernel_spmd(nc, in_maps, core_ids=list(range(NCORES)), trace=trace)
    out = np.empty((M, N), dtype=np.float32)
    for c in range(NCORES):
        out[:, c * NS : (c + 1) * NS] = res.results[c]["out"]
    return out, res


def kernel(**inputs) -> np.ndarray:
    out, _ = run(inputs, trace=False)
    return out


# revision 23
# speedup vs baseline: 1.0406x; 1.0136x over previous
"""WOQ int4 linear (4096x4096x11008, G=128) + bias + silu + mul on 8 NeuronCores.

Column-parallel: N=11008 is split into 8 shards of 1376; inp is replicated.
Per core: dequantize the int4 weight shard to bf16 in SBUF once, transpose
inp tiles on the tensor engine, then a bf16 matmul accumulating in PSUM with
the bias folded in as a rank-1 matmul, and a fused silu*mul epilogue.

kernel(**inputs) takes the FULL inputs (as produced by the problem's
setup_inputs) and returns the FULL [4096, 11008] float32 output.
"""

import sys
from contextlib import ExitStack

if "/opt/trn_rl_repo" not in sys.path:
    sys.path.insert(0, "/opt/trn_rl_repo")

import numpy as np

import concourse.bacc as bacc
import concourse.mybir as mybir
import concourse.tile as tile
from concourse.bass_utils import run_bass_kernel_spmd
from concourse.masks import make_identity

FP32 = mybir.dt.float32
BF16 = mybir.dt.bfloat16
I32 = mybir.dt.int32
I16 = mybir.dt.int16

M, K, N = 4096, 4096, 11008
NCORES = 8
NS = N // NCORES          # 1376 columns per core
NW = NS // 8              # 172 packed int32 words per row
G = 128


def _n_chunks(NS, step=512):
    out, off = [], 0
    while off < NS:
        ln = min(step, NS - off)
        out.append((off, ln))
        off += ln
    return out


def _build_kernel(ctx: ExitStack, tc: tile.TileContext, io: dict):
    nc = tc.nc
    inp, qweight, scales, qzeros, bias, mul, out = (
        io["inp"], io["qweight"], io["woq_scales"], io["woq_qzeros"],
        io["woq_bias"], io["mul"], io["out"],
    )
    M, K = inp.shape
    NS = out.shape[1]
    NW = NS // 8
    KT = K // 128             # k-tiles == quant groups (G == 128)
    MT = M // 128
    KH = K // 2               # inp is staged in row halves
    CH = _n_chunks(NS)

    const = ctx.enter_context(tc.tile_pool(name="const", bufs=1))
    wpool = ctx.enter_context(tc.tile_pool(name="wpool", bufs=KT))
    prep = ctx.enter_context(tc.tile_pool(name="prep", bufs=1))
    deq = ctx.enter_context(tc.tile_pool(name="deq", bufs=3))
    bcast = ctx.enter_context(tc.tile_pool(name="bcast", bufs=3))
    qwpool = ctx.enter_context(tc.tile_pool(name="qwpool", bufs=3))
    mloop = ctx.enter_context(tc.tile_pool(name="mloop", bufs=3))
    ep = ctx.enter_context(tc.tile_pool(name="ep", bufs=2))
    inpool = ctx.enter_context(tc.tile_pool(name="inpool", bufs=2))
    dram = ctx.enter_context(tc.tile_pool(name="dram", bufs=1, space="DRAM"))
    psum_mm_pool = ctx.enter_context(tc.tile_pool(name="psmm", bufs=2, space="PSUM"))
    psum_tp_pool = ctx.enter_context(tc.tile_pool(name="pstp", bufs=2, space="PSUM"))

    # constants
    identity = const.tile([128, 128], FP32)
    make_identity(nc, identity)
    ones_col = const.tile([1, 128], BF16)
    nc.vector.memset(ones_col[:], 1.0)

    # ---- phase A: zeros/scales prep ----
    s_sb = prep.tile([KT, NS], FP32)
    nc.sync.dma_start(s_sb[:], scales[:, :])
    qz_sb = prep.tile([KT, NW], I32)
    nc.sync.dma_start(qz_sb[:], qzeros[:, :])
    bias_f32 = prep.tile([1, NS], FP32)
    nc.sync.dma_start(bias_f32[:], bias[:, :])
    bias_bf = prep.tile([1, NS], BF16)
    nc.vector.tensor_copy(bias_bf[:], bias_f32[:])

    z_i32 = deq.tile([KT, NW, 8], I32, tag="wint")
    for j in range(8):
        nc.vector.tensor_scalar(
            out=z_i32[:, :, j], in0=qz_sb[:],
            scalar1=4 * j, scalar2=0xF,
            op0=mybir.AluOpType.logical_shift_right,
            op1=mybir.AluOpType.bitwise_and,
        )
    zs = prep.tile([KT, NS], FP32)
    nc.vector.tensor_copy(zs[:], z_i32.rearrange("p a b -> p (a b)"))
    nc.vector.tensor_tensor(
        out=zs[:], in0=zs[:], in1=s_sb[:], op=mybir.AluOpType.mult
    )
    s_bf = prep.tile([KT, NS], BF16)
    nc.vector.tensor_copy(s_bf[:], s_sb[:])
    zs_bf = prep.tile([KT, NS], BF16)
    nc.vector.tensor_copy(zs_bf[:], zs[:])
    s_d = dram.tile([KT, NS], BF16)
    nc.sync.dma_start(s_d[:], s_bf[:])
    zs_d = dram.tile([KT, NS], BF16)
    nc.sync.dma_start(zs_d[:], zs_bf[:])

    # ---- phase B: dequantize weight shard into SBUF, bf16, [k_p, n_f] ----
    # w[k, n] = wint[k, n] * s[g, n] - z[g, n] * s[g, n]
    w_sb = [wpool.tile([128, NS], BF16, tag="w", name=f"w_sb{i}") for i in range(KT)]
    for g in range(KT):
        s_b = bcast.tile([128, NS], BF16, tag="s_b")
        nc.sync.dma_start(s_b[:], s_d[g : g + 1, :].broadcast_to([128, NS]))
        zs_b = bcast.tile([128, NS], BF16, tag="zs_b")
        nc.sync.dma_start(zs_b[:], zs_d[g : g + 1, :].broadcast_to([128, NS]))

        qw_t = qwpool.tile([128, NW], I32, tag="qw")
        nc.sync.dma_start(qw_t[:], qweight[g * 128 : (g + 1) * 128, :])
        wint = deq.tile([128, NW, 8], I32, tag="wint")
        for j in range(8):
            nc.vector.tensor_scalar(
                out=wint[:, :, j], in0=qw_t[:],
                scalar1=4 * j, scalar2=0xF,
                op0=mybir.AluOpType.logical_shift_right,
                op1=mybir.AluOpType.bitwise_and,
            )
        nc.vector.tensor_tensor(
            out=w_sb[g][:], in0=wint.rearrange("p a b -> p (a b)"), in1=s_b[:],
            op=mybir.AluOpType.mult,
        )
        nc.vector.tensor_tensor(
            out=w_sb[g][:], in0=w_sb[g][:], in1=zs_b[:], op=mybir.AluOpType.subtract
        )

    # ---- phase C: m-tile pairs: transpose inp, interleaved matmuls, epilogue ----
    assert MT % 2 == 0
    for mp in range(0, MT, 2):
        inpTs = []
        for mi in (mp, mp + 1):
            inpT = mloop.tile([128, KT, 128], BF16, tag="inpT", name=f"inpT{mi}")
            inpTs.append(inpT)
            for half in range(2):
                inp_t = inpool.tile([128, KH], FP32, tag="inp")
                nc.scalar.dma_start(
                    inp_t[:], inp[mi * 128 : (mi + 1) * 128, half * KH : (half + 1) * KH]
                )
                kt0 = half * (KT // 2)
                for kt4 in range(0, KT // 2, 4):
                    ng = min(4, KT // 2 - kt4)
                    ps_tp = psum_tp_pool.tile([128, 512], FP32, tag="tp")
                    for q in range(ng):
                        nc.tensor.transpose(
                            ps_tp[:, q * 128 : (q + 1) * 128],
                            inp_t[:, (kt4 + q) * 128 : (kt4 + q + 1) * 128],
                            identity[:],
                        )
                    dst = inpT[:, kt0 + kt4 : kt0 + kt4 + ng, :].rearrange("p a b -> p (a b)")
                    nc.scalar.copy(dst, ps_tp[:, : ng * 128])

        pss = []
        for mi in (mp, mp + 1):
            ps = psum_mm_pool.tile([128, NS], FP32, tag="mm", name=f"ps{mi}")
            pss.append(ps)
            for off, ln in CH:
                nc.tensor.matmul(
                    ps[:, off : off + ln], ones_col[:], bias_bf[:, off : off + ln],
                    start=True, stop=False,
                )
        for kt in range(KT):
            last = kt == KT - 1
            for ps, inpT in zip(pss, inpTs):
                for off, ln in CH:
                    nc.tensor.matmul(
                        ps[:, off : off + ln],
                        inpT[:, kt, :],
                        w_sb[kt][:, off : off + ln],
                        start=False, stop=last,
                    )

        for ps, mi in zip(pss, (mp, mp + 1)):
            mul_t = ep.tile([128, NS], FP32, tag="mul")
            nc.scalar.dma_start(mul_t[:], mul[mi * 128 : (mi + 1) * 128, :])
            silu_t = ep.tile([128, NS], FP32, tag="silu")
            nc.scalar.activation(silu_t[:], ps[:], mybir.ActivationFunctionType.Silu)
            nc.vector.tensor_tensor(
                out=silu_t[:], in0=silu_t[:], in1=mul_t[:], op=mybir.AluOpType.mult
            )
            nc.sync.dma_start(out[mi * 128 : (mi + 1) * 128, :], silu_t[:])


_NC_CACHE = None


def _get_nc():
    global _NC_CACHE
    if _NC_CACHE is not None:
        return _NC_CACHE
    nc = bacc.Bacc("TRN2", target_bir_lowering=False, debug=False, num_devices=NCORES)
    KT = K // 128
    io = {
        "inp": nc.dram_tensor("inp", [M, K], FP32, kind="ExternalInput").ap(),
        "qweight": nc.dram_tensor("qweight", [K, NW], I32, kind="ExternalInput").ap(),
        "woq_scales": nc.dram_tensor("woq_scales", [KT, NS], FP32, kind="ExternalInput").ap(),
        "woq_qzeros": nc.dram_tensor("woq_qzeros", [KT, NW], I32, kind="ExternalInput").ap(),
        "woq_bias": nc.dram_tensor("woq_bias", [1, NS], FP32, kind="ExternalInput").ap(),
        "mul": nc.dram_tensor("mul", [M, NS], FP32, kind="ExternalInput").ap(),
        "out": nc.dram_tensor("out", [M, NS], FP32, kind="ExternalOutput").ap(),
    }
    with tile.TileContext(nc) as tc:
        with ExitStack() as ctx:
            _build_kernel(ctx, tc, io)
    nc.compile()
    _NC_CACHE = nc
    return nc


def _shard_inputs(inp, qweight, woq_scales, woq_qzeros, woq_bias, mul):
    in_maps = []
    for c in range(NCORES):
        n0, nw0 = c * NS, c * NW
        in_maps.append({
            "inp": inp,
            "qweight": np.ascontiguousarray(qweight[:, nw0 : nw0 + NW]),
            "woq_scales": np.ascontiguousarray(woq_scales[:, n0 : n0 + NS]),
            "woq_qzeros": np.ascontiguousarray(woq_qzeros[:, nw0 : nw0 + NW]),
            "woq_bias": np.ascontiguousarray(woq_bias.reshape(1, N)[:, n0 : n0 + NS]),
            "mul": np.ascontiguousarray(mul[:, n0 : n0 + NS]),
        })
    return in_maps


def run(inputs: dict, trace: bool = False):
    inp = np.asarray(inputs["inp"], dtype=np.float32)
    qweight = np.asarray(inputs["qweight"], dtype=np.int32)
    woq_scales = np.asarray(inputs["woq_scales"], dtype=np.float32)
    woq_qzeros = np.asarray(inputs["woq_qzeros"], dtype=np.int32)
    woq_bias = np.asarray(inputs["woq_bias"], dtype=np.float32)
    mul = np.asarray(inputs["mul"], dtype=np.float32)
    assert int(inputs.get("group_size", G)) == G
    assert inp.shape == (M, K) and qweight.shape == (K, N // 8)

    nc = _get_nc()
    in_maps = _shard_inputs(inp, qweight, woq_scales, woq_qzeros, woq_bias, mul)
    res = run_bass_kernel_spmd(nc, in_maps, core_ids=list(range(NCORES)), trace=trace)
    out = np.empty((M, N), dtype=np.float32)
    for c in range(NCORES):
        out[:, c * NS : (c + 1) * NS] = res.results[c]["out"]
    return out, res


def kernel(**inputs) -> np.ndarray:
    out, _ = run(inputs, trace=False)
    return out


# revision 24
# speedup vs baseline: 1.0577x; 1.0164x over previous
"""WOQ int4 linear (4096x4096x11008, G=128) + bias + silu + mul on 8 NeuronCores.

Column-parallel: N=11008 is split into 8 shards of 1376; inp is replicated.
Per core: dequantize the int4 weight shard to bf16 in SBUF once, transpose
inp tiles on the tensor engine, then a bf16 matmul accumulating in PSUM with
the bias folded in as a rank-1 matmul, and a fused silu*mul epilogue.

kernel(**inputs) takes the FULL inputs (as produced by the problem's
setup_inputs) and returns the FULL [4096, 11008] float32 output.
"""

import sys
from contextlib import ExitStack

if "/opt/trn_rl_repo" not in sys.path:
    sys.path.insert(0, "/opt/trn_rl_repo")

import numpy as np

import concourse.bacc as bacc
import concourse.mybir as mybir
import concourse.tile as tile
from concourse.bass_utils import run_bass_kernel_spmd
from concourse.masks import make_identity

FP32 = mybir.dt.float32
BF16 = mybir.dt.bfloat16
I32 = mybir.dt.int32
I16 = mybir.dt.int16

M, K, N = 4096, 4096, 11008
NCORES = 8
NS = N // NCORES          # 1376 columns per core
NW = NS // 8              # 172 packed int32 words per row
G = 128


def _n_chunks(NS, step=512):
    out, off = [], 0
    while off < NS:
        ln = min(step, NS - off)
        out.append((off, ln))
        off += ln
    return out


def _build_kernel(ctx: ExitStack, tc: tile.TileContext, io: dict):
    nc = tc.nc
    inp, qweight, scales, qzeros, bias, mul, out = (
        io["inp"], io["qweight"], io["woq_scales"], io["woq_qzeros"],
        io["woq_bias"], io["mul"], io["out"],
    )
    M, K = inp.shape
    NS = out.shape[1]
    NW = NS // 8
    KT = K // 128             # k-tiles == quant groups (G == 128)
    MT = M // 128
    KH = K // 2               # inp is staged in row halves
    CH = _n_chunks(NS)

    const = ctx.enter_context(tc.tile_pool(name="const", bufs=1))
    wpool = ctx.enter_context(tc.tile_pool(name="wpool", bufs=KT))
    prep = ctx.enter_context(tc.tile_pool(name="prep", bufs=1))
    deq = ctx.enter_context(tc.tile_pool(name="deq", bufs=3))
    bcast = ctx.enter_context(tc.tile_pool(name="bcast", bufs=3))
    qwpool = ctx.enter_context(tc.tile_pool(name="qwpool", bufs=3))
    mloop = ctx.enter_context(tc.tile_pool(name="mloop", bufs=3))
    ep = ctx.enter_context(tc.tile_pool(name="ep", bufs=2))
    inpool = ctx.enter_context(tc.tile_pool(name="inpool", bufs=2))
    dram = ctx.enter_context(tc.tile_pool(name="dram", bufs=1, space="DRAM"))
    psum_mm_pool = ctx.enter_context(tc.tile_pool(name="psmm", bufs=6, space="PSUM"))
    psum_tp_pool = ctx.enter_context(tc.tile_pool(name="pstp", bufs=2, space="PSUM"))

    # constants
    identity = const.tile([128, 128], FP32)
    make_identity(nc, identity)
    ones_col = const.tile([1, 128], BF16)
    nc.vector.memset(ones_col[:], 1.0)

    # ---- phase A: zeros/scales prep ----
    s_sb = prep.tile([KT, NS], FP32)
    nc.sync.dma_start(s_sb[:], scales[:, :])
    qz_sb = prep.tile([KT, NW], I32)
    nc.sync.dma_start(qz_sb[:], qzeros[:, :])
    bias_f32 = prep.tile([1, NS], FP32)
    nc.sync.dma_start(bias_f32[:], bias[:, :])
    bias_bf = prep.tile([1, NS], BF16)
    nc.vector.tensor_copy(bias_bf[:], bias_f32[:])

    z_i32 = deq.tile([KT, NW, 8], I32, tag="wint")
    for j in range(8):
        nc.vector.tensor_scalar(
            out=z_i32[:, :, j], in0=qz_sb[:],
            scalar1=4 * j, scalar2=0xF,
            op0=mybir.AluOpType.logical_shift_right,
            op1=mybir.AluOpType.bitwise_and,
        )
    zs = prep.tile([KT, NS], FP32)
    nc.vector.tensor_copy(zs[:], z_i32.rearrange("p a b -> p (a b)"))
    nc.vector.tensor_tensor(
        out=zs[:], in0=zs[:], in1=s_sb[:], op=mybir.AluOpType.mult
    )
    s_bf = prep.tile([KT, NS], BF16)
    nc.vector.tensor_copy(s_bf[:], s_sb[:])
    zs_bf = prep.tile([KT, NS], BF16)
    nc.vector.tensor_copy(zs_bf[:], zs[:])
    s_d = dram.tile([KT, NS], BF16)
    nc.sync.dma_start(s_d[:], s_bf[:])
    zs_d = dram.tile([KT, NS], BF16)
    nc.sync.dma_start(zs_d[:], zs_bf[:])

    # ---- phase B: dequantize weight shard into SBUF, bf16, [k_p, n_f] ----
    # w[k, n] = wint[k, n] * s[g, n] - z[g, n] * s[g, n]
    w_sb = [wpool.tile([128, NS], BF16, tag="w", name=f"w_sb{i}") for i in range(KT)]
    for g in range(KT):
        s_b = bcast.tile([128, NS], BF16, tag="s_b")
        nc.sync.dma_start(s_b[:], s_d[g : g + 1, :].broadcast_to([128, NS]))
        zs_b = bcast.tile([128, NS], BF16, tag="zs_b")
        nc.sync.dma_start(zs_b[:], zs_d[g : g + 1, :].broadcast_to([128, NS]))

        qw_t = qwpool.tile([128, NW], I32, tag="qw")
        nc.sync.dma_start(qw_t[:], qweight[g * 128 : (g + 1) * 128, :])
        wint = deq.tile([128, NW, 8], I32, tag="wint")
        for j in range(8):
            nc.vector.tensor_scalar(
                out=wint[:, :, j], in0=qw_t[:],
                scalar1=4 * j, scalar2=0xF,
                op0=mybir.AluOpType.logical_shift_right,
                op1=mybir.AluOpType.bitwise_and,
            )
        nc.vector.tensor_tensor(
            out=w_sb[g][:], in0=wint.rearrange("p a b -> p (a b)"), in1=s_b[:],
            op=mybir.AluOpType.mult,
        )
        nc.vector.tensor_tensor(
            out=w_sb[g][:], in0=w_sb[g][:], in1=zs_b[:], op=mybir.AluOpType.subtract
        )

    # ---- phase C: m-tile pairs: transpose inp, interleaved matmuls, epilogue ----
    assert MT % 2 == 0
    for mp in range(0, MT, 2):
        inpTs = []
        for mi in (mp, mp + 1):
            inpT = mloop.tile([128, KT, 128], BF16, tag="inpT", name=f"inpT{mi}")
            inpTs.append(inpT)
            for half in range(2):
                inp_t = inpool.tile([128, KH], FP32, tag="inp")
                nc.scalar.dma_start(
                    inp_t[:], inp[mi * 128 : (mi + 1) * 128, half * KH : (half + 1) * KH]
                )
                kt0 = half * (KT // 2)
                for kt4 in range(0, KT // 2, 4):
                    ng = min(4, KT // 2 - kt4)
                    ps_tp = psum_tp_pool.tile([128, 512], FP32, tag="tp")
                    for q in range(ng):
                        nc.tensor.transpose(
                            ps_tp[:, q * 128 : (q + 1) * 128],
                            inp_t[:, (kt4 + q) * 128 : (kt4 + q + 1) * 128],
                            identity[:],
                        )
                    dst = inpT[:, kt0 + kt4 : kt0 + kt4 + ng, :].rearrange("p a b -> p (a b)")
                    nc.scalar.copy(dst, ps_tp[:, : ng * 128])

        pss = []
        for mi in (mp, mp + 1):
            row = []
            for ci, (off, ln) in enumerate(CH):
                pc = psum_mm_pool.tile([128, 512], FP32, tag="mm", name=f"ps{mi}_{ci}")
                row.append(pc)
                nc.tensor.matmul(
                    pc[:, :ln], ones_col[:], bias_bf[:, off : off + ln],
                    start=True, stop=False,
                )
            pss.append(row)
        for kt in range(KT):
            last = kt == KT - 1
            for row, inpT in zip(pss, inpTs):
                for ci, (off, ln) in enumerate(CH):
                    nc.tensor.matmul(
                        row[ci][:, :ln],
                        inpT[:, kt, :],
                        w_sb[kt][:, off : off + ln],
                        start=False, stop=last,
                    )

        for row, mi in zip(pss, (mp, mp + 1)):
            mul_t = ep.tile([128, NS], FP32, tag="mul")
            nc.scalar.dma_start(mul_t[:], mul[mi * 128 : (mi + 1) * 128, :])
            silu_t = ep.tile([128, NS], FP32, tag="silu")
            for ci, (off, ln) in enumerate(CH):
                nc.scalar.activation(
                    silu_t[:, off : off + ln], row[ci][:, :ln],
                    mybir.ActivationFunctionType.Silu,
                )
                nc.vector.tensor_tensor(
                    out=silu_t[:, off : off + ln], in0=silu_t[:, off : off + ln],
                    in1=mul_t[:, off : off + ln], op=mybir.AluOpType.mult,
                )
            nc.sync.dma_start(out[mi * 128 : (mi + 1) * 128, :], silu_t[:])


_NC_CACHE = None


def _get_nc():
    global _NC_CACHE
    if _NC_CACHE is not None:
        return _NC_CACHE
    nc = bacc.Bacc("TRN2", target_bir_lowering=False, debug=False, num_devices=NCORES)
    KT = K // 128
    io = {
        "inp": nc.dram_tensor("inp", [M, K], FP32, kind="ExternalInput").ap(),
        "qweight": nc.dram_tensor("qweight", [K, NW], I32, kind="ExternalInput").ap(),
        "woq_scales": nc.dram_tensor("woq_scales", [KT, NS], FP32, kind="ExternalInput").ap(),
        "woq_qzeros": nc.dram_tensor("woq_qzeros", [KT, NW], I32, kind="ExternalInput").ap(),
        "woq_bias": nc.dram_tensor("woq_bias", [1, NS], FP32, kind="ExternalInput").ap(),
        "mul": nc.dram_tensor("mul", [M, NS], FP32, kind="ExternalInput").ap(),
        "out": nc.dram_tensor("out", [M, NS], FP32, kind="ExternalOutput").ap(),
    }
    with tile.TileContext(nc) as tc:
        with ExitStack() as ctx:
            _build_kernel(ctx, tc, io)
    nc.compile()
    _NC_CACHE = nc
    return nc


def _shard_inputs(inp, qweight, woq_scales, woq_qzeros, woq_bias, mul):
    in_maps = []
    for c in range(NCORES):
        n0, nw0 = c * NS, c * NW
        in_maps.append({
            "inp": inp,
            "qweight": np.ascontiguousarray(qweight[:, nw0 : nw0 + NW]),
            "woq_scales": np.ascontiguousarray(woq_scales[:, n0 : n0 + NS]),
            "woq_qzeros": np.ascontiguousarray(woq_qzeros[:, nw0 : nw0 + NW]),
            "woq_bias": np.ascontiguousarray(woq_bias.reshape(1, N)[:, n0 : n0 + NS]),
            "mul": np.ascontiguousarray(mul[:, n0 : n0 + NS]),
        })
    return in_maps


def run(inputs: dict, trace: bool = False):
    inp = np.asarray(inputs["inp"], dtype=np.float32)
    qweight = np.asarray(inputs["qweight"], dtype=np.int32)
    woq_scales = np.asarray(inputs["woq_scales"], dtype=np.float32)
    woq_qzeros = np.asarray(inputs["woq_qzeros"], dtype=np.int32)
    woq_bias = np.asarray(inputs["woq_bias"], dtype=np.float32)
    mul = np.asarray(inputs["mul"], dtype=np.float32)
    assert int(inputs.get("group_size", G)) == G
    assert inp.shape == (M, K) and qweight.shape == (K, N // 8)

    nc = _get_nc()
    in_maps = _shard_inputs(inp, qweight, woq_scales, woq_qzeros, woq_bias, mul)
    res = run_bass_kernel_spmd(nc, in_maps, core_ids=list(range(NCORES)), trace=trace)
    out = np.empty((M, N), dtype=np.float32)
    for c in range(NCORES):
        out[:, c * NS : (c + 1) * NS] = res.results[c]["out"]
    return out, res


def kernel(**inputs) -> np.ndarray:
    out, _ = run(inputs, trace=False)
    return out
